# revision 18
# baseline (speedup 1.0000x reference)
"""Trainium2 Bass kernel for CSWin-style full attention with LePE.

Module (B=2, C=256, H=W=48, heads=8, head_dim=32):
    qkv = conv1x1(x)            -> q, k, v per head
    attn = softmax(k^T q * d^-0.5, over keys)
    out  = v @ attn + lepe(v)   (lepe = depthwise 3x3 conv + bias)
    out  = conv1x1(out) + b_proj

Sharding: 16 (batch, head) units over 8 cores -> each core owns one batch
index and two heads (A, B); the host sums the 4 partial projections per
batch and adds b_proj.

v3 design:
  - S = k^T q in PE row-tiled groups of 3 k-tiles (tile_position=(32i,0),
    32x128 mode), q/k DMA-replicated at 4 partition bases.  Two 3-bank
    PSUM sets (banks 0-2 / 3-5) ping-pong so each set's
    S-write -> exp-read chain stays off the critical path.
  - exp split between ScalarE (activation, [128, 3*512] multi-bank PSUM
    reads) and DVE (Schraudolph bits = rne(s*K0 + K1) -> int16 IS fp16
    2^y, one stock tensor_scalar, ~3% max rel err which cancels between
    numerator and the ones-column denominator).
  - PV col-tiled (128x64 mode): unit A accumulates in PSUM partitions
    0:33, unit B in 64:97 of bank 6, concurrently; vT carries a ones
    column so the softmax denominator drops out of the same matmuls.
  - The PE queue is in-order, so PV(c-1) / v / vT / lepe / proj(c-1)
    blocks are woven between S groups to fill the exp-wait stalls (keeps
    HAM at K=8/8 and hides all non-S PE work behind the exp wall).
  - lepe: merged block-diagonal 9-tap stream over zero-padded v (A ->
    out 0:32, B -> 64:96), bias folded into the DVE evacuation.
  - Normalization: dens (PSUM rows 32/96) -> partitions 0/64, custom-DVE
    reciprocal, one group-local stream_shuffle broadcast, multiply out
    of PV PSUM, lepe add on GpSimd.  PV junk rows zeroed once.
"""

from collections import deque

import numpy as np

import concourse.bacc as bacc
import concourse.mybir as mybir
import concourse.tile as tile
from concourse.bass_utils import run_bass_kernel_spmd

F16 = mybir.dt.float16
F32 = mybir.dt.float32
I16 = mybir.dt.int16
ADD = mybir.AluOpType.add
MULT = mybir.AluOpType.mult
EXP = mybir.ActivationFunctionType.Exp

B, C, H, W = 2, 256, 48, 48
N = H * W                      # 2304
HEADS, D = 8, 32
SCALE = D ** -0.5
NCORES = 8
KT = N // 128                  # 18 key tiles
TAPS = [(dy, dx) for dy in (-1, 0, 1) for dx in (-1, 0, 1)]

K0 = SCALE * 1.4426950408889634 * 1024.0
K1 = (15.0 - 0.043033) * 1024.0

CHUNKS = [(0, 512), (512, 512), (1024, 512), (1536, 512), (2048, 256)]
SGROUPS = [(0, 3), (3, 3), (6, 3), (9, 3), (12, 3), (15, 3)]
ENG = {0: "AVAVAV", 1: "AAVAVA"}


def _chunks(total, step):
    out, o = [], 0
    while o < total:
        out.append((o, min(step, total - o)))
        o += step
    return out


def _emit(nc, tc, pools, tensors, dbg=None):
    const, sb, obp, ps_a, ps_b, ps_v, ps_o = pools
    x_d, wqk_d, wv_d, dg_d, bl_d, wp_d, out_d = tensors

    # ---- persistent SBUF tensors -----------------------------------
    x_sb = sb.tile([128, 2, N], F16, tag="x")
    qk_tmp = sb.tile([128, N], F16, tag="qktmp")
    qrep = sb.tile([128, 2, N], F16, tag="qrep")
    krep = sb.tile([128, 2, N], F16, tag="krep")
    vpad = sb.tile([128, 50, 50], F16, tag="vpad")      # rows 64:128 zero
    vsb = sb.tile([64, N], F16, tag="vsb")
    # contiguous DMA-transpose target: [:, kt, 0:32]=vA, [32:64]=vB
    vABt = sb.tile([128, KT, 64], F16, tag="vABt")
    # per-unit weight slabs [vT | ones]
    vT2A = sb.tile([128, KT, 33], F16, tag="vT2A")
    vT2B = sb.tile([128, KT, 33], F16, tag="vT2B")
    lepe128 = sb.tile([128, N], F16, tag="lepe")        # A 0:32, B 64:96
    p_t = [[sb.tile([128, KT, 512], F16, tag=f"p{u}{pb}",
                    name=f"p{u}{pb}") for pb in range(2)] for u in range(2)]
    y3 = [sb.tile([128, 512], F16, tag=f"y{i}", name=f"y{i}")
          for i in range(2)]
    dn128 = sb.tile([128, 512], F32, tag="dn")
    rc128 = sb.tile([128, 512], F32, tag="rc")
    rbs = sb.tile([128, 512], F32, tag="rbs")
    tm = sb.tile([128, 512], F16, tag="tm")

    wqk = const.tile([128, 2, 128], F16, tag="wqk")
    wv = const.tile([128, 2, 128], F16, tag="wv")       # cols 64:128 zero
    dg = const.tile([128, 9, 128], F16, tag="dg")       # merged block-diag
    bl = const.tile([128, 1], F32, tag="bl")
    wp = const.tile([128, 2, 128], F16, tag="wp")

    for cc in range(2):
        nc.sync.dma_start(wqk[:, cc, :], wqk_d[cc])
    for h0, hw in _chunks(N, 576):
        for cc in range(2):
            nc.sync.dma_start(x_sb[:, cc, h0:h0 + hw], x_d[cc, :, h0:h0 + hw])
    nc.sync.dma_start(wv[:, :, :], wv_d[:, :, :])
    nc.sync.dma_start(dg[:, :, :], dg_d[:, :, :])
    nc.sync.dma_start(bl[:, :], bl_d[:, :])
    nc.sync.dma_start(wp[:, :, :], wp_d[:, :, :])

    nc.vector.memset(vpad[:], 0.0)
    nc.vector.memset(vT2A[:, :, 32:33], 1.0)
    nc.vector.memset(vT2B[:, :, 32:33], 1.0)
    nc.vector.memset(dn128[:], 1.0)

    # ---- PSUM layout (8 banks exactly) -----------------------------
    sA = ps_a.tile([128, 3, 512], F32, tag="sA")        # banks 0-2
    sB = ps_b.tile([128, 3, 512], F32, tag="sB")        # banks 3-5
    pv = ps_v.tile([128, 512], F32, tag="pv")           # bank 6
    pso = ps_o.tile([128, 512], F32, tag="pso")         # bank 7
    # junk rows of the PV bank must be finite zeros (read by the epilogue
    # multiply; proj weights are zero there)
    nc.vector.memset(pv[32:64, :], 0.0)
    nc.vector.memset(pv[96:128, :], 0.0)

    ph0 = [sA[:, i, :] for i in range(3)] + [sB[:, i, :] for i in range(3)]
    slot = [0]

    def scratch():
        s = ph0[slot[0] % 6]
        slot[0] += 1
        return s

    # ---- phase 0: qkv + replication only ---------------------------
    for c0, cwq in _chunks(N, 512):
        t = scratch()
        nc.tensor.matmul(t[:, :cwq], wqk[:, 0, :], x_sb[:, 0, c0:c0 + cwq],
                         start=True, stop=False)
        nc.tensor.matmul(t[:, :cwq], wqk[:, 1, :], x_sb[:, 1, c0:c0 + cwq],
                         start=False, stop=True)
        nc.scalar.copy(qk_tmp[:, c0:c0 + cwq], t[:, :cwq])
    # qk_tmp rows: qA 0:32 | kA 32:64 | qB 64:96 | kB 96:128
    for u in range(2):
        eng = nc.sync if u == 0 else nc.scalar
        for i in range(4):
            eng.dma_start(qrep[32 * i:32 * i + 32, u, :],
                          qk_tmp[64 * u:64 * u + 32, :])
            eng.dma_start(krep[32 * i:32 * i + 32, u, :],
                          qk_tmp[64 * u + 32:64 * u + 64, :])

    # ---- deferred PE blocks, woven into the exp-wait stalls --------
    bg_q = deque()
    pv_q = deque()

    def v_block(r0, nr):
        def blk():
            c0, cwv = r0 * W, nr * W
            t = pso
            nc.tensor.matmul(t[:, :cwv], wv[:, 0, :], x_sb[:, 0, c0:c0 + cwv],
                             start=True, stop=False)
            nc.tensor.matmul(t[:, :cwv], wv[:, 1, :], x_sb[:, 1, c0:c0 + cwv],
                             start=False, stop=True)
            nc.vector.tensor_copy(vpad[0:64, 1 + r0:1 + r0 + nr, 1:49],
                                  t[0:64, :cwv])
            nc.scalar.copy(vsb[0:64, c0:c0 + cwv], t[0:64, :cwv])
        return blk

    def vt_dma():
        # vT via the DMA xbar transpose engine: no PE/DVE cost at all
        nc.sync.dma_start_transpose(vABt[:, :, :], vsb[0:64, :])
        # strided SBUF->SBUF DMAs build the per-unit weight slabs
        nc.sync.dma_start(vT2A[:, :, 0:32], vABt[:, :, 0:32])
        nc.sync.dma_start(vT2B[:, :, 0:32], vABt[:, :, 32:64])

    def lepe_block(r0, nr):
        def blk():
            t = pso
            for ti, (dy, dx) in enumerate(TAPS):
                nc.tensor.matmul(
                    t[:, :nr * W], dg[:, ti, :],
                    vpad[:, 1 + r0 + dy:1 + r0 + dy + nr, 1 + dx:49 + dx],
                    start=(ti == 0), stop=(ti == 8))
            nc.vector.tensor_scalar(lepe128[:, r0 * W:(r0 + nr) * W],
                                    t[:, :nr * W], bl[:, 0:1], None, ADD)
        return blk

    for blk in ([v_block(r0, nr) for r0, nr in _chunks(H, 10)] + [vt_dma]
                + [lepe_block(r0, nr) for r0, nr in _chunks(H, 10)]):
        bg_q.append(blk)

    def pv_block(ci, kts):
        q0, cw = CHUNKS[ci]

        def blk():
            for kt in kts:
                nc.tensor.matmul(pv[0:33, :cw], vT2A[:, kt, 0:33],
                                 p_t[0][ci % 2][:, kt, :cw],
                                 start=(kt == 0), stop=(kt == KT - 1),
                                 tile_position=(0, 0))
                nc.tensor.matmul(pv[64:97, :cw], vT2B[:, kt, 0:33],
                                 p_t[1][ci % 2][:, kt, :cw],
                                 start=(kt == 0), stop=(kt == KT - 1),
                                 tile_position=(0, 64))
        return blk

    def epi(ci):
        q0, cw = CHUNKS[ci]
        nc.vector.tensor_copy(dn128[0:1, :cw], pv[32:33, :cw])
        nc.scalar.copy(dn128[64:65, :cw], pv[96:97, :cw])
        nc.vector.reciprocal_approx_fast(rc128[:, :cw], dn128[:, :cw])
        nc.vector.stream_shuffle(rbs[:, :cw], rc128[:, :cw], [0] * 32)
        nc.vector.tensor_mul(tm[:, :cw], pv[:, :cw], rbs[:, :cw])
        nc.gpsimd.tensor_add(y3[ci % 2][:, :cw], tm[:, :cw],
                             lepe128[:, q0:q0 + cw])

    def proj_block(ci):
        q0, cw = CHUNKS[ci]

        def blk():
            for mc in range(2):
                nc.tensor.matmul(pso[:, :cw], wp[:, mc, :],
                                 y3[ci % 2][:, :cw],
                                 start=True, stop=True)
                ob = obp.tile([128, 512], F32, tag="ob")
                nc.scalar.copy(ob[:, :cw], pso[:, :cw])
                nc.sync.dma_start(out_d[mc, :, q0:q0 + cw], ob[:, :cw])
        return blk

    def drain(q, n):
        for _ in range(min(n, len(q))):
            q.popleft()()

    # ---- attention chunks ------------------------------------------
    for ci, (q0, cw) in enumerate(CHUNKS):
        if ci > 0:
            for k0 in range(0, KT, 3):
                pv_q.append(pv_block(ci - 1, list(range(k0, k0 + 3))))
        for u in range(2):
            pt = p_t[u][ci % 2]
            for gi, (kt0, nk) in enumerate(SGROUPS):
                ps = sA if gi % 2 == 0 else sB
                for j in range(nk):
                    kt = kt0 + j
                    nc.tensor.matmul(
                        ps[:, j, :cw],
                        krep[32 * j:32 * j + 32, u, kt * 128:(kt + 1) * 128],
                        qrep[32 * j:32 * j + 32, u, q0:q0 + cw],
                        start=True, stop=True, tile_position=(32 * j, 0))
                if ENG[u][gi] == "A":
                    nc.scalar.activation(pt[:, kt0:kt0 + nk, :cw],
                                         ps[:, 0:nk, :cw], EXP, scale=SCALE)
                else:
                    nc.vector.tensor_scalar(
                        pt[:, kt0:kt0 + nk, :cw].bitcast(I16),
                        ps[:, 0:nk, :cw], K0, K1, MULT, ADD)
                # weave: PV of the previous chunk (ready, no waits)
                if pv_q:
                    drain(pv_q, 1)
                else:
                    drain(bg_q, 2 if ci == 0 else 1)
        drain(pv_q, KT)
        if ci > 0:
            epi(ci - 1)
            proj_block(ci - 1)()
    last = len(CHUNKS) - 1
    for k0 in range(0, KT, 3):
        pv_q.append(pv_block(last, list(range(k0, k0 + 3))))
    drain(pv_q, KT)
    drain(bg_q, 99)
    epi(last)
    proj_block(last)()
    if dbg is not None:
        nc.sync.dma_start(dbg["vsb"][:, :], vsb[:, :])
        nc.sync.dma_start(dbg["vT2A"][:, :, :], vT2A[:, :, :])
        nc.sync.dma_start(dbg["vT2B"][:, :, :], vT2B[:, :, :])
        nc.sync.dma_start(dbg["lepe"][:, :], lepe128[:, :])
        nc.sync.dma_start(dbg["qrep"][:, :, :], qrep[:, :, :])
        nc.sync.dma_start(dbg["krep"][:, :, :], krep[:, :, :])
        nc.sync.dma_start(dbg["p00"][:, :, :], p_t[0][0][:, :, :])
        nc.sync.dma_start(dbg["p10"][:, :, :], p_t[1][0][:, :, :])


def _build():
    nc = bacc.Bacc("TRN2", target_bir_lowering=False, debug=False)

    x_d = nc.dram_tensor("x", [2, 128, N], F16, kind="ExternalInput")
    wqk_d = nc.dram_tensor("wqk", [2, 128, 128], F16, kind="ExternalInput")
    wv_d = nc.dram_tensor("wv", [128, 2, 128], F16, kind="ExternalInput")
    dg_d = nc.dram_tensor("dg", [128, 9, 128], F16, kind="ExternalInput")
    bl_d = nc.dram_tensor("bl", [128, 1], F32, kind="ExternalInput")
    wp_d = nc.dram_tensor("wp", [128, 2, 128], F16, kind="ExternalInput")
    out_d = nc.dram_tensor("out", [2, 128, N], F32, kind="ExternalOutput")
    dbg = None
    import os
    if os.environ.get("KDBG"):
        dbg = {
            "vsb": nc.dram_tensor("dbg_vsb", [64, N], F16, kind="ExternalOutput"),
            "vT2A": nc.dram_tensor("dbg_vT2A", [128, KT, 33], F16, kind="ExternalOutput"),
            "vT2B": nc.dram_tensor("dbg_vT2B", [128, KT, 33], F16, kind="ExternalOutput"),
            "lepe": nc.dram_tensor("dbg_lepe", [128, N], F16, kind="ExternalOutput"),
            "qrep": nc.dram_tensor("dbg_qrep", [128, 2, N], F16, kind="ExternalOutput"),
            "krep": nc.dram_tensor("dbg_krep", [128, 2, N], F16, kind="ExternalOutput"),
            "p00": nc.dram_tensor("dbg_p00", [128, KT, 512], F16, kind="ExternalOutput"),
            "p10": nc.dram_tensor("dbg_p10", [128, KT, 512], F16, kind="ExternalOutput"),
        }

    with tile.TileContext(nc) as tc:
        with (
            tc.tile_pool(name="const", bufs=1) as const,
            tc.tile_pool(name="sb", bufs=1) as sb,
            tc.tile_pool(name="ob", bufs=4) as obp,
            tc.tile_pool(name="ps_a", bufs=1, space="PSUM") as ps_a,
            tc.tile_pool(name="ps_b", bufs=1, space="PSUM") as ps_b,
            tc.tile_pool(name="ps_v", bufs=1, space="PSUM") as ps_v,
            tc.tile_pool(name="ps_o", bufs=1, space="PSUM") as ps_o,
        ):
            _emit(nc, tc, (const, sb, obp, ps_a, ps_b, ps_v, ps_o),
                  (x_d, wqk_d, wv_d, dg_d, bl_d, wp_d, out_d), dbg=dbg)

    nc.compile()
    return nc


_NC = None


def _get_nc():
    global _NC
    if _NC is None:
        _NC = _build()
    return _NC


def _prep_core(c, x, w_qkv, w_lepe, b_lepe, w_proj):
    b = c // 4
    hA, hB = 2 * (c % 4), 2 * (c % 4) + 1
    xb = np.asarray(x[b], np.float32).reshape(C, N)
    w_qkv = np.asarray(w_qkv, np.float32)
    w_lepe = np.asarray(w_lepe, np.float32)
    b_lepe = np.asarray(b_lepe, np.float32)
    w_proj = np.asarray(w_proj, np.float32)

    rows = np.concatenate([
        w_qkv[96 * hA + 0:96 * hA + 32],       # qA
        w_qkv[96 * hA + 32:96 * hA + 64],      # kA
        w_qkv[96 * hB + 0:96 * hB + 32],       # qB
        w_qkv[96 * hB + 32:96 * hB + 64],      # kB
    ], axis=0)                                 # [128, 256]
    wqk = np.ascontiguousarray(rows.T.reshape(2, 128, 128)).astype(np.float16)

    # wv[c', cc, j]: v weights for both units, transposed; cols 64:128 zero
    wv = np.zeros((2, 128, 128), np.float32)
    wv[:, :, 0:32] = w_qkv[96 * hA + 64:96 * hA + 96].T.reshape(2, 128, 32)
    wv[:, :, 32:64] = w_qkv[96 * hB + 64:96 * hB + 96].T.reshape(2, 128, 32)
    wv = np.ascontiguousarray(wv.transpose(1, 0, 2)).astype(np.float16)

    # merged block-diag lepe: A vpad rows 0:32 -> out 0:32, B rows
    # 32:64 -> out 64:96
    dg = np.zeros((128, 9, 128), np.float32)
    idx = np.arange(32)
    for ti, (dy, dx) in enumerate(TAPS):
        dg[idx, ti, idx] = w_lepe[32 * hA:32 * hA + 32, 0, dy + 1, dx + 1]
        dg[32 + idx, ti, 64 + idx] = w_lepe[32 * hB:32 * hB + 32, 0,
                                            dy + 1, dx + 1]
    dg = dg.astype(np.float16)

    bl = np.zeros((128, 1), np.float32)
    bl[0:32, 0] = b_lepe[32 * hA:32 * hA + 32]
    bl[64:96, 0] = b_lepe[32 * hB:32 * hB + 32]

    # wp[c', mc, o']: proj weights; y rows A 0:32, B 64:96, rest zero
    wp = np.zeros((128, 2, 128), np.float32)
    wp[0:32] = w_proj[:, 32 * hA:32 * hA + 32].T.reshape(32, 2, 128)
    wp[64:96] = w_proj[:, 32 * hB:32 * hB + 32].T.reshape(32, 2, 128)
    wp = wp.astype(np.float16)

    return {
        "x": np.ascontiguousarray(xb.reshape(2, 128, N)).astype(np.float16),
        "wqk": wqk, "wv": wv, "dg": dg, "bl": bl, "wp": wp,
    }


_LAST_RES = None


def kernel(x, w_qkv, w_lepe, b_lepe, w_proj, b_proj, **_ignored):
    global _LAST_RES
    nc = _get_nc()
    in_maps = [_prep_core(c, x, w_qkv, w_lepe, b_lepe, w_proj)
               for c in range(NCORES)]
    res = run_bass_kernel_spmd(nc, in_maps, core_ids=list(range(NCORES)))
    _LAST_RES = res
    out = np.zeros((B, C, N), np.float32)
    for c in range(NCORES):
        out[c // 4] += res.results[c]["out"].reshape(C, N)
    out += np.asarray(b_proj, np.float32)[None, :, None]
    return out.reshape(B, C, H, W).astype(np.float32)
